# revision 5
# baseline (speedup 1.0000x reference)
"""Trainium2 Bass kernel for fused multi-head causal attention.

Module: out = o_proj(causal_attention(rope_swapped(qkv_proj(x)))).
Shapes: x [2, 2048, 2048], 16 heads, head_dim 128.

Sharding (8 cores): batch (2) x head-group (4 groups of 4 heads).
Each core computes qkv projection + attention for its 4 heads of its
batch, then a partial o_proj against its slice of w_o rows.  The
all-reduce after o_proj is done host-side by summing the 4 partials
per batch (mathematically identical, avoids device collectives).

Device-kernel design notes:
 - QKV projection computes Q^T/K^T/V^T ([head_dim, S] layout) directly:
   out = W_slice.T @ x^T, so attention's QK^T matmul needs no transposes.
 - Q/K head pairs are interleaved ([q_ha_lo | q_hb_lo] on 128 partitions)
   so RoPE's rotate_half partner lives at the SAME partition of a sibling
   tile -> full-width DVE ops (3 ops/element, no cross-partition shuffle).
   The QK^T matmul then becomes 2 concurrent K=64 row-group matmuls.
 - Scores are computed transposed ([sk, sq]) so the PV matmul consumes
   exp(scores) directly; softmax denominators come from a ones-vector
   matmul; normalization is folded into the PSUM->SBUF copy of attn^T.
 - Causal masking via tensor_mask_reduce (per-partition mask_start).
 - All matmul inputs bf16 (1 cycle/row on PE), fp32 PSUM accumulation.
"""

import math
import os

import ml_dtypes
import numpy as np

S = 2048
D = 2048
HD = 128
NH = 16
N_CORES = 8
SQ = 512          # free-dim chunk for matmuls / psum tiles
NJ = S // SQ      # 4 s-chunks
KT = D // 128     # 16 contraction chunks
NST = S // 128    # 16 s-tiles of 128
BF16 = ml_dtypes.bfloat16

_MODULE_CACHE = {}


def _build_module():
    from contextlib import ExitStack

    import concourse.bass as bass
    import concourse.bacc as bacc
    import concourse.mybir as mybir
    import concourse.tile as tile

    f32 = mybir.dt.float32
    bf16 = mybir.dt.bfloat16
    ts = bass.ts

    nc = bacc.Bacc("TRN2", target_bir_lowering=False, debug=False,
                   num_devices=N_CORES)

    # DRAM I/O (identical program on all cores; per-core data differs)
    xT = nc.dram_tensor("xT", [D, S], bf16, kind="ExternalInput").ap()
    wq = nc.dram_tensor("wq", [D, 12 * 128], bf16, kind="ExternalInput").ap()
    bq = nc.dram_tensor("bq", [1, 12 * 128], bf16, kind="ExternalInput").ap()
    wo = nc.dram_tensor("wo", [4 * 128, D], bf16, kind="ExternalInput").ap()
    stab_d = nc.dram_tensor("stab", [128, S], bf16, kind="ExternalInput").ap()
    ctab_d = nc.dram_tensor("ctab", [128, S], bf16, kind="ExternalInput").ap()
    cmask_d = nc.dram_tensor("cmask", [128, 4 * SQ], bf16, kind="ExternalInput").ap()
    ident_d = nc.dram_tensor("ident", [128, 128], bf16, kind="ExternalInput").ap()
    out_d = nc.dram_tensor("out", [S, D], f32, kind="ExternalOutput").ap()

    inv_sqrt_hd = 1.0 / math.sqrt(HD)

    with tile.TileContext(nc) as tc, ExitStack() as ctx:
        # Long-lived pools first; xt/w/tab live in an inner scope that is
        # closed after the last QKV pass so the wo pool can reuse the space
        # (SBUF pool allocation is a LIFO stack).
        ps = ctx.enter_context(
            tc.tile_pool(name="ps", bufs=8, space=bass.MemorySpace.PSUM))
        const_p = ctx.enter_context(tc.tile_pool(name="const", bufs=1))
        qk_p = ctx.enter_context(tc.tile_pool(name="qk", bufs=8))
        vt_p = ctx.enter_context(tc.tile_pool(name="vt", bufs=1))
        v_p = ctx.enter_context(tc.tile_pool(name="v", bufs=3))
        scr_p = ctx.enter_context(tc.tile_pool(name="scr", bufs=4))
        p_p = ctx.enter_context(tc.tile_pool(name="p", bufs=6))
        attn_p = ctx.enter_context(tc.tile_pool(name="attn", bufs=4))
        rcp_p = ctx.enter_context(tc.tile_pool(name="rcp", bufs=2))
        bc_p = ctx.enter_context(tc.tile_pool(name="bc", bufs=2))
        out_p = ctx.enter_context(tc.tile_pool(name="outp", bufs=2))
        ctx2 = ctx.enter_context(ExitStack())
        xt_p = ctx2.enter_context(tc.tile_pool(name="xt", bufs=KT))
        w_p = ctx2.enter_context(tc.tile_pool(name="w", bufs=KT))
        tab_p = ctx2.enter_context(tc.tile_pool(name="tab", bufs=2))

        # --- constants / tables -------------------------------------------
        stab = tab_p.tile([128, S], bf16, tag="tab")
        nc.sync.dma_start(out=stab[:], in_=stab_d[:])
        ctab = tab_p.tile([128, S], bf16, tag="tab")
        nc.sync.dma_start(out=ctab[:], in_=ctab_d[:])
        cmask = const_p.tile([128, 4 * SQ], bf16, tag="c0")
        nc.sync.dma_start(out=cmask[:], in_=cmask_d[:])
        ident = const_p.tile([128, 128], bf16, tag="c1")
        nc.sync.dma_start(out=ident[:], in_=ident_d[:])
        bias_sb = const_p.tile([1, 12 * 128], bf16, tag="c2")
        nc.sync.dma_start(out=bias_sb[:], in_=bq[:])
        ones_row = const_p.tile([1, SQ], bf16, tag="c4")
        nc.vector.memset(ones_row[:], 1.0)
        ones_col = const_p.tile([128, 1], bf16, tag="c5")
        nc.vector.memset(ones_col[:], 1.0)
        ones_f32 = const_p.tile([1, 128], f32, tag="c6")
        nc.vector.memset(ones_f32[:], 1.0)

        # x^T resident in SBUF (read by both pair phases)
        xt = []
        for k in range(KT):
            t = xt_p.tile([128, S], bf16, tag="xt")
            nc.sync.dma_start(out=t[:], in_=xT[k * 128:(k + 1) * 128, :])
            xt.append(t)

        def load_pair_w(pair):
            tiles = []
            for k in range(KT):
                t = w_p.tile([128, 6 * 128], bf16, tag="w")
                nc.sync.dma_start(
                    out=t[:],
                    in_=wq[k * 128:(k + 1) * 128,
                           pair * 768:(pair + 1) * 768])
                tiles.append(t)
            return tiles

        def qkv_pass(w_tiles, pair, mats, js):
            """Accumulate x @ W + b for the given mats (within-pair indices)
            and s-chunks.  Returns {(j, m): psum tile [128, SQ] fp32}."""
            accs = {}
            for j in js:
                for m in mats:
                    accs[(j, m)] = ps.tile([128, SQ], f32, tag="ps", name="qkv_acc")
            for k in range(KT):
                for j in js:
                    for m in mats:
                        nc.tensor.matmul(
                            accs[(j, m)][:],
                            w_tiles[k][:, ts(m, 128)],
                            xt[k][:, ts(j, SQ)],
                            start=(k == 0), stop=False)
            gm = pair * 6
            for j in js:
                for m in mats:
                    nc.tensor.matmul(
                        accs[(j, m)][:],
                        bias_sb[0:1, (gm + m) * 128:(gm + m + 1) * 128],
                        ones_row[0:1, :],
                        start=False, stop=True)
            return accs

        def rope(j, A, B, dstA, dstB):
            # dstA = A*sin - B*cos ; dstB = B*sin + A*cos   (full-width)
            sl = stab[:, ts(j, SQ)]
            cl = ctab[:, ts(j, SQ)]
            t1 = scr_p.tile([128, SQ], bf16, tag="scr")
            nc.vector.tensor_mul(t1[:], A[:], sl)
            t2 = scr_p.tile([128, SQ], bf16, tag="scr")
            nc.vector.tensor_mul(t2[:], B[:], cl)
            nc.vector.tensor_sub(dstA, t1[:], t2[:])
            t3 = scr_p.tile([128, SQ], bf16, tag="scr")
            nc.vector.tensor_mul(t3[:], B[:], sl)
            t4 = scr_p.tile([128, SQ], bf16, tag="scr")
            nc.vector.tensor_mul(t4[:], A[:], cl)
            nc.vector.tensor_add(dstB, t3[:], t4[:])

        attnT = []   # 4 head tiles [128, S] bf16, in head order

        for pair in range(2):
            w_tiles = load_pair_w(pair)

            # Q/K projection (+bias) and rope, pair-interleaved layout
            qlo = qk_p.tile([128, S], bf16, tag="qk")
            qhi = qk_p.tile([128, S], bf16, tag="qk")
            klo = qk_p.tile([128, S], bf16, tag="qk")
            khi = qk_p.tile([128, S], bf16, tag="qk")
            for js in ((0, 1), (2, 3)):
                accs = qkv_pass(w_tiles, pair, (0, 1, 2, 3), js)
                for j in js:
                    rope(j, accs[(j, 0)], accs[(j, 1)],
                         qlo[:, ts(j, SQ)], qhi[:, ts(j, SQ)])
                    rope(j, accs[(j, 2)], accs[(j, 3)],
                         klo[:, ts(j, SQ)], khi[:, ts(j, SQ)])

            # V projection -> V^T -> transpose to [s, d] layout per head
            vaccs = qkv_pass(w_tiles, pair, (4, 5), (0, 1, 2, 3))
            vs = []
            for hh in range(2):
                vt = vt_p.tile([128, S], bf16, tag="vt")
                for j in range(NJ):
                    nc.any.tensor_copy(vt[:, ts(j, SQ)], vaccs[(j, 4 + hh)][:])
                v = v_p.tile([128, S], bf16, tag="v")
                for t in range(NST):
                    pt = ps.tile([128, 128], bf16, tag="ps")
                    nc.tensor.transpose(pt[:], vt[:, ts(t, 128)], ident[:])
                    nc.any.tensor_copy(v[:, ts(t, 128)], pt[:])
                vs.append(v)

            # attention for the pair's two heads, processed jointly
            aT = [attn_p.tile([128, S], bf16, tag="attn", name="aT") for _ in range(2)]
            for j in range(NJ):
                ndiag = 4 * j + 4
                apsum = [ps.tile([128, SQ], f32, tag="ps", name="apsum") for _ in range(2)]
                dpsum = [ps.tile([1, SQ], f32, tag="ps", name="dpsum") for _ in range(2)]
                for i in range(ndiag):
                    lg = [ps.tile([128, SQ], f32, tag="ps", name="lg") for _ in range(2)]
                    sl0 = slice(0, 64)
                    sl1 = slice(64, 128)
                    # logits^T[sk,sq] = K^T.T @ Q^T, split lo/hi head-dim
                    # halves; the two heads run in disjoint row groups.
                    nc.tensor.matmul(lg[0][:], klo[sl0, ts(i, 128)],
                                     qlo[sl0, ts(j, SQ)], start=True, stop=False)
                    nc.tensor.matmul(lg[1][:], klo[sl1, ts(i, 128)],
                                     qlo[sl1, ts(j, SQ)], start=True, stop=False)
                    nc.tensor.matmul(lg[0][:], khi[sl0, ts(i, 128)],
                                     qhi[sl0, ts(j, SQ)], start=False, stop=True)
                    nc.tensor.matmul(lg[1][:], khi[sl1, ts(i, 128)],
                                     qhi[sl1, ts(j, SQ)], start=False, stop=True)
                    r = i - 4 * j
                    last = (i == ndiag - 1)
                    for hh in range(2):
                        if r >= 0:
                            # causal: add -9e15 where sq < 128*r + p
                            nc.vector.tensor_add(
                                lg[hh][:], lg[hh][:], cmask[:, ts(r, SQ)])
                        p_t = p_p.tile([128, SQ], bf16, tag="p")
                        nc.scalar.activation(
                            p_t[:], lg[hh][:],
                            mybir.ActivationFunctionType.Exp,
                            scale=inv_sqrt_hd)
                        nc.tensor.matmul(dpsum[hh][:], ones_col[:, 0:1],
                                         p_t[:], start=(i == 0), stop=last)
                        nc.tensor.matmul(apsum[hh][:],
                                         vs[hh][:, ts(i, 128)], p_t[:],
                                         start=(i == 0), stop=last)
                for hh in range(2):
                    rc = rcp_p.tile([1, SQ], f32, tag="rcp")
                    nc.vector.reciprocal(rc[:], dpsum[hh][:])
                    # broadcast 1/denom across partitions via a K=1 matmul
                    bcp = ps.tile([128, SQ], f32, tag="ps", name="bcp")
                    nc.tensor.matmul(bcp[:], ones_f32[0:1, :], rc[0:1, :],
                                     start=True, stop=True)
                    bc = bc_p.tile([128, SQ], f32, tag="bc")
                    nc.any.tensor_copy(bc[:], bcp[:])
                    nc.vector.tensor_mul(aT[hh][:, ts(j, SQ)],
                                         apsum[hh][:], bc[:])
            attnT.extend(aT)

        # o_proj partial: out[s, :] = sum_h attn_h[s, :] @ wo_h
        ctx2.close()   # release xt/w/tab space; wo reuses it
        wo_p = ctx.enter_context(tc.tile_pool(name="wo", bufs=4))
        wo_sb = []
        for hh in range(4):
            t = wo_p.tile([128, D], bf16, tag="wo")
            nc.sync.dma_start(out=t[:], in_=wo[hh * 128:(hh + 1) * 128, :])
            wo_sb.append(t)
        for st in range(NST):
            ops = [ps.tile([128, SQ], f32, tag="ps", name="oproj") for _ in range(4)]
            for hh in range(4):
                for e in range(4):
                    nc.tensor.matmul(ops[e][:],
                                     attnT[hh][:, ts(st, 128)],
                                     wo_sb[hh][:, ts(e, SQ)],
                                     start=(hh == 0), stop=(hh == 3))
            for e in range(4):
                ot = out_p.tile([128, SQ], f32, tag="outp")
                nc.any.tensor_copy(ot[:], ops[e][:])
                nc.sync.dma_start(
                    out=out_d[st * 128:(st + 1) * 128, e * SQ:(e + 1) * SQ],
                    in_=ot[:])

    nc.compile()
    return nc


def _host_inputs(x, w_qkv, b_qkv, w_o):
    """Build the 8 per-core input maps."""
    x = np.asarray(x, dtype=np.float32)
    w_qkv = np.asarray(w_qkv, dtype=np.float32)
    b_qkv = np.asarray(b_qkv, dtype=np.float32)
    w_o = np.asarray(w_o, dtype=np.float32)

    # rope tables (reference swaps sin/cos roles; we follow the math:
    # q_rot = q*sin(emb) + rotate_half(q)*cos(emb))
    inv_freq = 1.0 / (10000.0 ** (np.arange(0, HD, 2, dtype=np.float32) / HD))
    t = np.arange(S, dtype=np.float32)
    freq = np.einsum("s,f->sf", t, inv_freq)          # [S, 64]
    sinT = np.sin(freq).T.astype(np.float32)          # [64, S]
    cosT = np.cos(freq).T.astype(np.float32)
    stab = np.concatenate([sinT, sinT], 0).astype(BF16)   # [128, S]
    ctab = np.concatenate([cosT, cosT], 0).astype(BF16)

    p_idx = np.arange(128)[:, None]
    f_idx = np.arange(SQ)[None, :]
    cmask = np.concatenate(
        [np.where(f_idx >= 128 * r + p_idx, 0.0, -9e15) for r in range(4)],
        1).astype(BF16)
    ident = np.eye(128, dtype=np.float32).astype(BF16)

    def head_w(h):
        base = h * 3 * HD
        return (w_qkv[:, base:base + HD],
                w_qkv[:, base + HD:base + 2 * HD],
                w_qkv[:, base + 2 * HD:base + 3 * HD])

    def head_b(h):
        base = h * 3 * HD
        return (b_qkv[base:base + HD],
                b_qkv[base + HD:base + 2 * HD],
                b_qkv[base + 2 * HD:base + 3 * HD])

    in_maps = []
    for c in range(N_CORES):
        b = c // 4
        heads = [4 * (c % 4) + i for i in range(4)]
        xT = np.ascontiguousarray(x[b].T).astype(BF16)

        mats, bvec = [], []
        for pair in range(2):
            ha, hb = heads[2 * pair], heads[2 * pair + 1]
            wq_a, wk_a, wv_a = head_w(ha)
            wq_b, wk_b, wv_b = head_w(hb)
            bq_a, bk_a, bv_a = head_b(ha)
            bq_b, bk_b, bv_b = head_b(hb)
            mats += [
                np.concatenate([wq_a[:, :64], wq_b[:, :64]], 1),
                np.concatenate([wq_a[:, 64:], wq_b[:, 64:]], 1),
                np.concatenate([wk_a[:, :64], wk_b[:, :64]], 1),
                np.concatenate([wk_a[:, 64:], wk_b[:, 64:]], 1),
                wv_a, wv_b,
            ]
            bvec += [
                np.concatenate([bq_a[:64], bq_b[:64]]),
                np.concatenate([bq_a[64:], bq_b[64:]]),
                np.concatenate([bk_a[:64], bk_b[:64]]),
                np.concatenate([bk_a[64:], bk_b[64:]]),
                bv_a, bv_b,
            ]
        wq_all = np.concatenate(mats, 1).astype(BF16)          # [D, 1536]
        bq_all = np.concatenate(bvec)[None, :].astype(BF16)    # [1, 1536]
        wo_all = np.concatenate(
            [w_o[h * HD:(h + 1) * HD, :] for h in heads], 0).astype(BF16)

        in_maps.append({
            "xT": xT, "wq": wq_all, "bq": bq_all, "wo": wo_all,
            "stab": stab, "ctab": ctab, "cmask": cmask, "ident": ident,
        })
    return in_maps


def _run(in_maps, trace=False):
    from concourse.bass_utils import run_bass_kernel_spmd
    if "nc" not in _MODULE_CACHE:
        _MODULE_CACHE["nc"] = _build_module()
    nc = _MODULE_CACHE["nc"]
    return run_bass_kernel_spmd(nc, in_maps, core_ids=list(range(N_CORES)),
                                trace=trace)


def kernel(x, w_qkv, b_qkv, w_o, b_o, _trace=False, _return_res=False):
    in_maps = _host_inputs(x, w_qkv, b_qkv, w_o)
    res = _run(in_maps, trace=_trace)
    out = np.zeros((2, S, D), dtype=np.float32)
    for c in range(N_CORES):
        out[c // 4] += res.results[c]["out"]
    out += np.asarray(b_o, dtype=np.float32)[None, None, :]
    if _return_res:
        return out, res
    return out


# revision 16
# speedup vs baseline: 199.0274x; 199.0274x over previous
"""Trainium2 Bass kernel for fused multi-head causal attention.

Module: out = o_proj(causal_attention(rope_swapped(qkv_proj(x)))).
Shapes: x [2, 2048, 2048], 16 heads, head_dim 128.

Sharding (8 cores): batch (2) x head-group (4 groups of 4 heads).
Each core computes qkv projection + attention for its 4 heads of its
batch, then a partial o_proj against its slice of w_o rows.  The
all-reduce after o_proj is done host-side by summing the 4 partials
per batch (mathematically identical, avoids device collectives).

Device-kernel design notes:
 - QKV projection computes Q^T/K^T/V^T ([head_dim, S] layout) directly:
   out = W_slice.T @ x^T, so attention's QK^T matmul needs no transposes.
 - Q/K head pairs are interleaved ([q_ha_lo | q_hb_lo] on 128 partitions)
   so RoPE's rotate_half partner lives at the SAME partition of a sibling
   tile -> full-width DVE ops (3 ops/element, no cross-partition shuffle).
   The QK^T matmul then becomes 2 concurrent K=64 row-group matmuls.
 - Scores are computed transposed ([sk, sq]) so the PV matmul consumes
   exp(scores) directly; softmax denominators come from a ones-vector
   matmul; normalization is folded into the PSUM->SBUF copy of attn^T.
 - Causal masking: fully-masked columns of diagonal score tiles are
   skipped outright; the remaining width-128 triangle gets an additive
   -9e15 mask tile before the exp.
 - All matmul inputs bf16 (1 cycle/row on PE), fp32 PSUM accumulation.
"""

import math
import os

import ml_dtypes
import numpy as np

S = 2048
D = 2048
HD = 128
NH = 16
N_CORES = 8
SQ = 512          # free-dim chunk for matmuls / psum tiles
NJ = S // SQ      # 4 s-chunks
KT = D // 128     # 16 contraction chunks
NST = S // 128    # 16 s-tiles of 128
BF16 = ml_dtypes.bfloat16

_MODULE_CACHE = {}


def _build_module():
    from contextlib import ExitStack

    import concourse.bass as bass
    import concourse.bacc as bacc
    import concourse.mybir as mybir
    import concourse.tile as tile

    f32 = mybir.dt.float32
    bf16 = mybir.dt.bfloat16
    ts = bass.ts

    nc = bacc.Bacc("TRN2", target_bir_lowering=False, debug=False,
                   num_devices=N_CORES)

    # DRAM I/O (identical program on all cores; per-core data differs)
    xT = nc.dram_tensor("xT", [D, S], bf16, kind="ExternalInput").ap()
    wq = nc.dram_tensor("wq", [D, 12 * 128], bf16, kind="ExternalInput").ap()
    bq = nc.dram_tensor("bq", [1, 12 * 128], bf16, kind="ExternalInput").ap()
    wo = nc.dram_tensor("wo", [4 * 128, D], bf16, kind="ExternalInput").ap()
    stab_d = nc.dram_tensor("stab", [128, S], bf16, kind="ExternalInput").ap()
    ctab_d = nc.dram_tensor("ctab", [128, S], bf16, kind="ExternalInput").ap()
    cmask_d = nc.dram_tensor("cmask", [128, 128], bf16, kind="ExternalInput").ap()
    ident_d = nc.dram_tensor("ident", [128, 128], bf16, kind="ExternalInput").ap()
    out_d = nc.dram_tensor("out", [S, D], f32, kind="ExternalOutput").ap()

    inv_sqrt_hd = 1.0 / math.sqrt(HD)

    with tile.TileContext(nc) as tc, ExitStack() as ctx:
        # Long-lived pools first; xt/w/tab/vt live in an inner scope that is
        # closed after the last QKV pass so the wo pool can reuse the space
        # (SBUF pool allocation is a LIFO stack).
        ps = ctx.enter_context(
            tc.tile_pool(name="ps", bufs=8, space=bass.MemorySpace.PSUM))
        const_p = ctx.enter_context(tc.tile_pool(name="const", bufs=1))
        qk_p = ctx.enter_context(tc.tile_pool(name="qk", bufs=8))
        v_p = ctx.enter_context(tc.tile_pool(name="v", bufs=3))
        scr_p = ctx.enter_context(tc.tile_pool(name="scr", bufs=6))
        p_p = ctx.enter_context(tc.tile_pool(name="p", bufs=10))
        attn_p = ctx.enter_context(tc.tile_pool(name="attn", bufs=4))
        rcp_p = ctx.enter_context(tc.tile_pool(name="rcp", bufs=2))
        bc_p = ctx.enter_context(tc.tile_pool(name="bc", bufs=3))
        out_p = ctx.enter_context(tc.tile_pool(name="outp", bufs=4))
        ctx2 = ctx.enter_context(ExitStack())
        xt_p = ctx2.enter_context(tc.tile_pool(name="xt", bufs=2 * KT))
        w_p = ctx2.enter_context(tc.tile_pool(name="w", bufs=KT))
        tab_p = ctx2.enter_context(tc.tile_pool(name="tab", bufs=2))
        vt_p = ctx2.enter_context(tc.tile_pool(name="vt", bufs=1))

        # x^T resident in SBUF as half-S tiles; DMAs interleaved with the
        # pair-0 weights so the first QKV pass starts immediately and the
        # stream stays just ahead of PE consumption.
        w_tiles0 = []
        xt_a, xt_b = [], []
        HS = S // 2
        for k in range(KT):
            t = w_p.tile([128, 6 * 128], bf16, tag="w", name="w0")
            nc.sync.dma_start(out=t[:], in_=wq[k * 128:(k + 1) * 128, 0:768])
            w_tiles0.append(t)
            t = xt_p.tile([128, HS], bf16, tag="xt", name="xta")
            nc.sync.dma_start(out=t[:], in_=xT[k * 128:(k + 1) * 128, 0:HS])
            xt_a.append(t)
            t = xt_p.tile([128, HS], bf16, tag="xt", name="xtb")
            nc.sync.dma_start(out=t[:], in_=xT[k * 128:(k + 1) * 128, HS:S])
            xt_b.append(t)

        def xt_rhs(k, j):
            h = xt_a if j < 2 else xt_b
            return h[k][:, ts(j % 2, SQ)]

        # constants / tables (all consumed later than the first QKV pass)
        stab = tab_p.tile([128, S], bf16, tag="tab")
        nc.sync.dma_start(out=stab[:], in_=stab_d[:])
        ctab = tab_p.tile([128, S], bf16, tag="tab")
        nc.sync.dma_start(out=ctab[:], in_=ctab_d[:])
        cmask = const_p.tile([128, 128], bf16, tag="c0")
        nc.sync.dma_start(out=cmask[:], in_=cmask_d[:])
        ident = const_p.tile([128, 128], bf16, tag="c1")
        nc.sync.dma_start(out=ident[:], in_=ident_d[:])
        bias_sb = const_p.tile([1, 12 * 128], bf16, tag="c2")
        nc.sync.dma_start(out=bias_sb[:], in_=bq[:])
        ones_row = const_p.tile([1, SQ], bf16, tag="c4")
        nc.vector.memset(ones_row[:], 1.0)
        ones_col = const_p.tile([128, 1], bf16, tag="c5")
        nc.vector.memset(ones_col[:], 1.0)
        f16 = mybir.dt.float16
        ones_f16 = const_p.tile([1, 128], f16, tag="c6")
        nc.vector.memset(ones_f16[:], 1.0)

        def load_pair_w(pair):
            tiles = []
            for k in range(KT):
                t = w_p.tile([128, 6 * 128], bf16, tag="w")
                nc.sync.dma_start(
                    out=t[:],
                    in_=wq[k * 128:(k + 1) * 128,
                           pair * 768:(pair + 1) * 768])
                tiles.append(t)
            return tiles

        def qkv_pass(w_tiles, pair, jms):
            """Accumulate x @ W + b for the given (j, mat) pairs.
            Returns {(j, m): psum tile [128, SQ] fp32}."""
            accs = {}
            for (j, m) in jms:
                accs[(j, m)] = ps.tile([128, SQ], f32, tag="ps",
                                       name="qkv_acc")
            for k in range(KT):
                for (j, m) in jms:
                    nc.tensor.matmul(
                        accs[(j, m)][:],
                        w_tiles[k][:, ts(m, 128)],
                        xt_rhs(k, j),
                        start=(k == 0), stop=False)
            gm = pair * 6
            for (j, m) in jms:
                nc.tensor.matmul(
                    accs[(j, m)][:],
                    bias_sb[0:1, (gm + m) * 128:(gm + m + 1) * 128],
                    ones_row[0:1, :],
                    start=False, stop=True)
            return accs

        def rope(j, A, B, dsts):
            """A=[lo ha|lo hb], B=[hi ha|hi hb] pair-interleaved psum tiles;
            writes per-head contiguous rotated [128, SQ] slices into
            dsts[0] (head a) and dsts[1] (head b):
              rot_lo = lo*sin - hi*cos ; rot_hi = hi*sin + lo*cos."""
            sl = stab[:, ts(j, SQ)]
            cl = ctab[:, ts(j, SQ)]
            t1 = scr_p.tile([128, SQ], bf16, tag="scr")
            nc.vector.tensor_mul(t1[:], A[:], sl)
            t2 = scr_p.tile([128, SQ], bf16, tag="scr")
            nc.vector.tensor_mul(t2[:], B[:], cl)
            t3 = scr_p.tile([128, SQ], bf16, tag="scr")
            nc.vector.tensor_mul(t3[:], B[:], sl)
            t4 = scr_p.tile([128, SQ], bf16, tag="scr")
            nc.vector.tensor_mul(t4[:], A[:], cl)
            for hh in range(2):
                hs = slice(64 * hh, 64 * hh + 64)
                nc.vector.tensor_sub(dsts[hh][0:64, ts(j, SQ)],
                                     t1[hs, :], t2[hs, :])
                nc.vector.tensor_add(dsts[hh][64:128, ts(j, SQ)],
                                     t3[hs, :], t4[hs, :])

        attnT = []   # 4 head tiles [128, S] bf16, in head order

        for pair in range(2):
            w_tiles = w_tiles0 if pair == 0 else load_pair_w(pair)

            # Q/K projection (+bias) and rope -> per-head contiguous tiles
            qT = [qk_p.tile([128, S], bf16, tag="qk", name="qT")
                  for _ in range(2)]
            kT = [qk_p.tile([128, S], bf16, tag="qk", name="kT")
                  for _ in range(2)]
            for j in range(NJ):
                accs = qkv_pass(w_tiles, pair,
                                [(j, m) for m in range(4)])
                rope(j, accs[(j, 0)], accs[(j, 1)], qT)
                rope(j, accs[(j, 2)], accs[(j, 3)], kT)

            # V projection -> V^T -> transpose to [s, d] layout per head
            vacc_h = [qkv_pass(w_tiles, pair,
                                 [(j, 4 + hh) for j in range(NJ)])
                      for hh in range(2)]
            vs = []
            for hh in range(2):
                vt = vt_p.tile([128, S], bf16, tag="vt")
                for j in range(NJ):
                    nc.scalar.copy(vt[:, ts(j, SQ)], vacc_h[hh][(j, 4 + hh)][:])
                v = v_p.tile([128, S], bf16, tag="v")
                for t in range(NST):
                    pt = ps.tile([128, 128], bf16, tag="ps")
                    nc.tensor.transpose(pt[:], vt[:, ts(t, 128)], ident[:])
                    nc.vector.tensor_copy(v[:, ts(t, 128)], pt[:])
                vs.append(v)

            # attention for the pair's two heads, processed jointly
            aT = [attn_p.tile([128, S], bf16, tag="attn", name="aT")
                  for _ in range(2)]
            for j in range(NJ):
                ndiag = 4 * j + 4
                apsum = dp = None
                for i in range(ndiag):
                    r = i - 4 * j
                    # columns left of a diagonal tile's valid triangle are
                    # fully masked -> skip them in QK/exp/PV/denom entirely
                    off = 128 * r if r > 0 else 0
                    lg = [ps.tile([128, SQ], f32, tag="ps", name="lg")
                          for _ in range(2)]
                    # logits^T[sk,sq] = K^T.T @ Q^T
                    for hh in range(2):
                        nc.tensor.matmul(lg[hh][:, off:SQ],
                                         kT[hh][:, ts(i, 128)],
                                         qT[hh][:, j * SQ + off:(j + 1) * SQ],
                                         start=True, stop=True)
                    if i == 0:
                        # allocated after the first logits tiles so the next
                        # chunk's QK matmuls can start while the previous
                        # chunk is still normalizing
                        apsum = [ps.tile([128, SQ], f32, tag="ps",
                                         name="apsum") for _ in range(2)]
                        dpsum = ps.tile([33, SQ], f32, tag="ps",
                                        name="dpsum")
                        dp = [dpsum[0:1, :], dpsum[32:33, :]]
                    last = (i == ndiag - 1)
                    for hh in range(2):
                        if r >= 0:
                            # causal: mask the width-128 triangle at the
                            # diagonal (cols off..off+128)
                            nc.vector.tensor_add(
                                lg[hh][:, off:off + 128],
                                lg[hh][:, off:off + 128], cmask[:])
                        p_t = p_p.tile([128, SQ], bf16, tag="p")
                        nc.scalar.activation(
                            p_t[:, off:SQ], lg[hh][:, off:SQ],
                            mybir.ActivationFunctionType.Exp,
                            scale=inv_sqrt_hd)
                        nc.tensor.matmul(dp[hh][:, off:SQ],
                                         ones_col[:, 0:1],
                                         p_t[:, off:SQ],
                                         start=(i == 0), stop=last)
                        nc.tensor.matmul(apsum[hh][:, off:SQ],
                                         vs[hh][:, ts(i, 128)],
                                         p_t[:, off:SQ],
                                         start=(i == 0), stop=last)
                for hh in range(2):
                    # fp16 reciprocal: full-rate matmul dtype, 2^-11 relative
                    # precision is ample for softmax denominators
                    rc = rcp_p.tile([1, SQ], f16, tag="rcp")
                    with nc.allow_low_precision(reason="fp16 1/denom"):
                        nc.vector.reciprocal(rc[:], dp[hh])
                    # broadcast 1/denom across partitions via a K=1 matmul
                    bcp = ps.tile([128, SQ], f32, tag="ps", name="bcp")
                    nc.tensor.matmul(bcp[:], ones_f16[0:1, :], rc[0:1, :],
                                     start=True, stop=True)
                    bc = bc_p.tile([128, SQ], f32, tag="bc")
                    nc.scalar.copy(bc[:], bcp[:])
                    nc.vector.tensor_mul(aT[hh][:, ts(j, SQ)],
                                         apsum[hh][:], bc[:])
            attnT.extend(aT)

        # o_proj partial: out[s, :] = sum_h attn_h[s, :] @ wo_h
        ctx2.close()   # release xt/w/tab/vt space; wo reuses it
        wo_p = ctx.enter_context(tc.tile_pool(name="wo", bufs=4))
        wo_sb = []
        for hh in range(4):
            t = wo_p.tile([128, D], bf16, tag="wo")
            nc.sync.dma_start(out=t[:], in_=wo[hh * 128:(hh + 1) * 128, :])
            wo_sb.append(t)
        for st in range(NST):
            for eg in range(2):
                ops = [ps.tile([128, SQ], f32, tag="ps", name="oproj")
                       for _ in range(2)]
                for hh in range(4):
                    for ei in range(2):
                        e = 2 * eg + ei
                        nc.tensor.matmul(ops[ei][:],
                                         attnT[hh][:, ts(st, 128)],
                                         wo_sb[hh][:, ts(e, SQ)],
                                         start=(hh == 0), stop=(hh == 3))
                for ei in range(2):
                    e = 2 * eg + ei
                    ot = out_p.tile([128, SQ], f32, tag="outp")
                    nc.vector.tensor_copy(ot[:], ops[ei][:])
                    nc.sync.dma_start(
                        out=out_d[st * 128:(st + 1) * 128,
                                  e * SQ:(e + 1) * SQ],
                        in_=ot[:])

    nc.compile()
    return nc


def _host_inputs(x, w_qkv, b_qkv, w_o):
    """Build the 8 per-core input maps."""
    x = np.asarray(x, dtype=np.float32)
    w_qkv = np.asarray(w_qkv, dtype=np.float32)
    b_qkv = np.asarray(b_qkv, dtype=np.float32)
    w_o = np.asarray(w_o, dtype=np.float32)

    # rope tables (reference swaps sin/cos roles; we follow the math:
    # q_rot = q*sin(emb) + rotate_half(q)*cos(emb))
    inv_freq = 1.0 / (10000.0 ** (np.arange(0, HD, 2, dtype=np.float32) / HD))
    t = np.arange(S, dtype=np.float32)
    freq = np.einsum("s,f->sf", t, inv_freq)          # [S, 64]
    sinT = np.sin(freq).T.astype(np.float32)          # [64, S]
    cosT = np.cos(freq).T.astype(np.float32)
    stab = np.concatenate([sinT, sinT], 0).astype(BF16)   # [128, S]
    ctab = np.concatenate([cosT, cosT], 0).astype(BF16)

    p_idx = np.arange(128)[:, None]
    f_idx = np.arange(128)[None, :]
    cmask = np.where(f_idx >= p_idx, 0.0, -9e15).astype(BF16)
    ident = np.eye(128, dtype=np.float32).astype(BF16)

    def head_w(h):
        base = h * 3 * HD
        return (w_qkv[:, base:base + HD],
                w_qkv[:, base + HD:base + 2 * HD],
                w_qkv[:, base + 2 * HD:base + 3 * HD])

    def head_b(h):
        base = h * 3 * HD
        return (b_qkv[base:base + HD],
                b_qkv[base + HD:base + 2 * HD],
                b_qkv[base + 2 * HD:base + 3 * HD])

    in_maps = []
    for c in range(N_CORES):
        b = c // 4
        heads = [4 * (c % 4) + i for i in range(4)]
        xT = np.ascontiguousarray(x[b].T).astype(BF16)

        mats, bvec = [], []
        for pair in range(2):
            ha, hb = heads[2 * pair], heads[2 * pair + 1]
            wq_a, wk_a, wv_a = head_w(ha)
            wq_b, wk_b, wv_b = head_w(hb)
            bq_a, bk_a, bv_a = head_b(ha)
            bq_b, bk_b, bv_b = head_b(hb)
            mats += [
                np.concatenate([wq_a[:, :64], wq_b[:, :64]], 1),
                np.concatenate([wq_a[:, 64:], wq_b[:, 64:]], 1),
                np.concatenate([wk_a[:, :64], wk_b[:, :64]], 1),
                np.concatenate([wk_a[:, 64:], wk_b[:, 64:]], 1),
                wv_a, wv_b,
            ]
            bvec += [
                np.concatenate([bq_a[:64], bq_b[:64]]),
                np.concatenate([bq_a[64:], bq_b[64:]]),
                np.concatenate([bk_a[:64], bk_b[:64]]),
                np.concatenate([bk_a[64:], bk_b[64:]]),
                bv_a, bv_b,
            ]
        wq_all = np.concatenate(mats, 1).astype(BF16)          # [D, 1536]
        bq_all = np.concatenate(bvec)[None, :].astype(BF16)    # [1, 1536]
        wo_all = np.concatenate(
            [w_o[h * HD:(h + 1) * HD, :] for h in heads], 0).astype(BF16)

        in_maps.append({
            "xT": xT, "wq": wq_all, "bq": bq_all, "wo": wo_all,
            "stab": stab, "ctab": ctab, "cmask": cmask, "ident": ident,
        })
    return in_maps


def _run(in_maps, trace=False):
    from concourse.bass_utils import run_bass_kernel_spmd
    if "nc" not in _MODULE_CACHE:
        _MODULE_CACHE["nc"] = _build_module()
    nc = _MODULE_CACHE["nc"]
    return run_bass_kernel_spmd(nc, in_maps, core_ids=list(range(N_CORES)),
                                trace=trace)


def kernel(x, w_qkv, b_qkv, w_o, b_o, _trace=False, _return_res=False):
    in_maps = _host_inputs(x, w_qkv, b_qkv, w_o)
    res = _run(in_maps, trace=_trace)
    out = np.zeros((2, S, D), dtype=np.float32)
    for c in range(N_CORES):
        out[c // 4] += res.results[c]["out"]
    out += np.asarray(b_o, dtype=np.float32)[None, None, :]
    if _return_res:
        return out, res
    return out
